# revision 45
# baseline (speedup 1.0000x reference)
# ISTFT kernel for Trainium2 (8 NeuronCores, data-parallel over batch).
#
# Math: out[b, s] for s = 256*c + r (chunk c, offset r) is
#   out[c, r] = sum_{j=0..3} sum_C spec[C, c-j] * invbasis[C, 256*j + r]
# i.e. the overlap-add is folded into 4 shifted matmuls accumulating in PSUM.
# invbasis rows 513 (imag DC) and 1025 (imag Nyquist) are exactly zero, so the
# contraction packs to exactly 1024 = 8 chunks of 128 partitions:
#   packed rows 0..511    = real rows 0..511   (mag*cos(angle))
#   packed row 512        = real row 512 (Nyquist)
#   packed rows 513..1023 = imag freqs 1..511  (mag*sin(angle))
#
# All-fp16 datapath (same PE rate as bf16, 3 more mantissa bits, DVE 2x):
#   tb  = a*(1/2pi) + 1536        (fp16 magic rounding, ulp(1536)=1)
#   kk  = tb - 1536               (= round(a/2pi), exact small int)
#   red = a - 2pi*kk              (scalar_tensor_tensor, f32 internal)
#   sin = Sin(red*s)              (ACT; s=0.999999 keeps +-pi in the table)
#   cos = Sin(pi/2*s - s*|red|)   (|red| via fp16 sign-bit mask)
#   spec = (1*mag) * {sin,cos}    (scalar_tensor_tensor, fp16 2x)
# Row 512 (real-only) is computed once per batch up-front on a [16,128]
# reshape of the 2048-sample row, then patched into spec[4] partition 0 with
# small per-slice SBUF->SBUF DMAs.
#
# DMA: the host pre-interleaves mag/angle into a slice-blocked [128, 4*2048]
# layout (chunk-major within each column slice) so every per-slice input
# transfer is ONE DMA with 2-4 KB contiguous runs per partition -- ~full HBM
# efficiency and minimal ring latency. invbasis rides the gpsimd SWDGE ring
# and row-512/patches ride the scalar ring, so neither the sync input ring
# nor the ACT queue ever head-of-line blocks (those stalls cost v3/v4 ~10 us).
# A short burst of discarded matmuls on the first angle tile warms the PE
# clock (it runs at ~half speed for the first ~3 us of activity).
import numpy as np

import concourse.bacc as bacc
import concourse.mybir as mybir
import concourse.tile as tile
from concourse.bass_utils import run_bass_kernel_spmd

F32 = mybir.dt.float32
F16 = mybir.dt.float16
U16 = mybir.dt.uint16
ALU = mybir.AluOpType
ACTF = mybir.ActivationFunctionType

TWO_PI = 6.283185307179586
INV_2PI = 1.0 / TWO_PI
PI = 3.141592653589793
MAGIC16 = 1536.0  # 1.5 * 2**10: fp16 ulp 1 -> forces round-to-nearest-int
SIN_SCALE = 0.999999
COS_BIAS = PI / 2 * SIN_SCALE

B_LOCAL = 2      # batches per core (16 total / 8 cores)
T = 2048         # STFT frames
SLICES = [(0, 256), (256, 256), (512, 512), (1024, 512), (1536, 384), (1920, 128)]
N_CORES = 8


def _reg_const(nc, value, dtype=F32):
    key = (dtype, value)
    if key in nc.const_aps.aps:
        return
    t = nc.alloc_sbuf_tensor(f"const-{dtype.name}-{value}", [128, 1], dtype)
    nc.gpsimd.memset(t.ap(), value)
    nc.const_aps.aps[key] = t.ap()


def build_nc(repeat=1):
    nc = bacc.Bacc(target_bir_lowering=False)
    _reg_const(nc, COS_BIAS)
    nc.all_engine_barrier()
    mag = nc.declare_dram_parameter("mag", [B_LOCAL, 128, 4 * T], F16, isOutput=False)
    ang = nc.declare_dram_parameter("angle", [B_LOCAL, 128, 4 * T], F16, isOutput=False)
    m5d = nc.declare_dram_parameter("mag5", [B_LOCAL, 16, 128], F16, isOutput=False)
    a5d = nc.declare_dram_parameter("angle5", [B_LOCAL, 16, 128], F16, isOutput=False)
    invb = nc.declare_dram_parameter("invbasis", [1024, 1024], F16, isOutput=False)
    out = nc.declare_dram_parameter("out", [B_LOCAL, 523008], F16, isOutput=True)

    with tile.TileContext(nc) as tc:
        with (
            tc.tile_pool(name="const", bufs=1) as constp,
            tc.tile_pool(name="stage", bufs=4) as stagep,
            tc.tile_pool(name="spec", bufs=2) as specp,
            tc.tile_pool(name="work", bufs=3) as workp,
            tc.tile_pool(name="r5", bufs=2) as r5p,
            tc.tile_pool(name="osb", bufs=4) as osbp,
            tc.tile_pool(name="psum", bufs=8, space="PSUM") as psump,
        ):
            ib = [
                constp.tile([128, 1024], F16, tag=f"ib{q}", name=f"ib{q}")
                for q in range(8)
            ]
            for q in range(8):
                nc.gpsimd.dma_start(
                    out=ib[q][:, :], in_=invb[128 * q : 128 * (q + 1), :]
                )

            def ew_chain(at, sv_out, cv_out, shp):
                tb = workp.tile(shp, F16, tag="tb", name="tb")
                nc.vector.tensor_scalar(
                    out=tb, in0=at, scalar1=INV_2PI, scalar2=MAGIC16,
                    op0=ALU.mult, op1=ALU.add,
                )
                kk = workp.tile(shp, F16, tag="kk", name="kk")
                nc.vector.tensor_scalar_sub(kk, tb, MAGIC16)
                red = workp.tile(shp, F16, tag="red", name="red", bufs=6)
                nc.vector.scalar_tensor_tensor(
                    out=red, in0=kk, scalar=-TWO_PI, in1=at,
                    op0=ALU.mult, op1=ALU.add,
                )
                ared = workp.tile(shp, F16, tag="ared", name="ared", bufs=6)
                nc.vector.tensor_scalar(
                    out=ared.bitcast(U16), in0=red.bitcast(U16),
                    scalar1=0x7FFF, scalar2=None, op0=ALU.bitwise_and,
                )
                if sv_out is not None:
                    nc.scalar.activation(sv_out, red, ACTF.Sin, scale=SIN_SCALE)
                nc.scalar.activation(cv_out, ared, ACTF.Sin, scale=-SIN_SCALE,
                                     bias=COS_BIAS)

            def emit_group(b, spec, ct):
                """32 accumulating matmuls -> one [128,256] output tile -> DRAM."""
                ps = psump.tile([128, 256], F32, tag="ps", name="ps")
                c0 = 128 * ct
                mmi = 0
                for q in range(8):
                    for j in range(4):
                        nc.tensor.matmul(
                            out=ps[:, :],
                            lhsT=spec[q][:, c0 - j + 4 : c0 - j + 132],
                            rhs=ib[q][:, 256 * j : 256 * (j + 1)],
                            start=(mmi == 0),
                            stop=(mmi == 31),
                        )
                        mmi += 1
                ob = osbp.tile([128, 256], F16, tag="ob", name="ob")
                lo = 256 * (128 * ct - 4)
                if ct >= 13:
                    # tail groups: split drain across ACT+DVE and the output
                    # transfer across both HWDGE rings so the post-matmul
                    # drain latency halves
                    nc.scalar.activation(ob[0:64, :], ps[0:64, :], ACTF.Copy)
                    nc.vector.tensor_scalar_mul(ob[64:128, :], ps[64:128, :], 1.0)
                    hi_rows = 128 if ct < 15 else 127
                    nc.sync.dma_start(out=out[b, lo : lo + 16384],
                                      in_=ob[0:64, :])
                    nc.scalar.dma_start(
                        out=out[b, lo + 16384 : lo + 256 * hi_rows],
                        in_=ob[64:hi_rows, :])
                else:
                    if ct % 2 == 0:
                        nc.scalar.activation(ob[:, :], ps[:, :], ACTF.Copy)
                    else:
                        nc.vector.tensor_scalar_mul(ob[:, :], ps[:, :], 1.0)
                    eng = nc.sync if ct % 2 == 0 else nc.scalar
                    if ct == 0:
                        eng.dma_start(out=out[b, 0:31744], in_=ob[4:128, :])
                    else:
                        eng.dma_start(out=out[b, lo : lo + 32768], in_=ob[:, :])

            # sequenced input stream: each slice's transfer is gated on the
            # previous slice's angle tile through a 1-element DVE copy (WAW on
            # the tile corner). DMA engines fair-share bandwidth over all
            # queued transfers, so without this the first slice completes
            # only after ~its share of the whole 4.2 MB stream; chained, it
            # gets the full ring and lands ~6 us earlier. One-slice lookahead
            # keeps the stream ahead of compute.
            batches = [b for _ in range(repeat) for b in range(B_LOCAL)]
            seq = [(bi, si) for bi in range(len(batches))
                   for si in range(len(SLICES))]
            tiles = {}
            prev_sam = [None]

            def issue_input(k):
                if k >= len(seq):
                    return
                bi, si = seq[k]
                s2, W2 = SLICES[si]
                sam = stagep.tile([128, 4, W2], F16, tag="sam", name="sam",
                                  bufs=4)
                smm = stagep.tile([128, 4, W2], F16, tag="smm", name="smm",
                                  bufs=4)
                if prev_sam[0] is not None:
                    nc.vector.tensor_scalar_mul(
                        sam[0:1, 0:1, 0:1], prev_sam[0][0:1, 0:1, 0:1], 1.0)
                bb = batches[bi]
                if k <= 1:
                    # ramp slices: split each tensor into chunk-pair halves.
                    # Per-transfer DMA bandwidth is engine-fanout-limited
                    # (~83 GB/s), so two co-flowing halves land ~2x sooner
                    # and chain(0)/chain(1) only depend on the first half.
                    # (Halving ALL slices, or quarter-splitting slice 0,
                    # regresses: trigger issue is ~700 ns per 128-descriptor
                    # DMA and contends with the rest of the stream.)
                    h = 2 * W2
                    nc.sync.dma_start(out=sam[:, 0:2, :],
                                      in_=ang[bb, :, 4 * s2 : 4 * s2 + h])
                    nc.sync.dma_start(out=sam[:, 2:4, :],
                                      in_=ang[bb, :, 4 * s2 + h : 4 * s2 + 2 * h])
                    nc.sync.dma_start(out=smm[:, 0:2, :],
                                      in_=mag[bb, :, 4 * s2 : 4 * s2 + h])
                    nc.sync.dma_start(out=smm[:, 2:4, :],
                                      in_=mag[bb, :, 4 * s2 + h : 4 * s2 + 2 * h])
                else:
                    nc.sync.dma_start(out=sam[...], in_=ang[bb, :, 4 * s2 : 4 * (s2 + W2)])
                    nc.sync.dma_start(out=smm[...], in_=mag[bb, :, 4 * s2 : 4 * (s2 + W2)])
                tiles[k] = (sam, smm)
                prev_sam[0] = sam

            issue_input(0)

            for bi, b in enumerate(batches):
                # row-512 inputs: tiny transfers on the scalar ring
                m5 = r5p.tile([16, 128], F16, tag="m5", name="m5")
                a5 = r5p.tile([16, 128], F16, tag="a5", name="a5")
                nc.scalar.dma_start(out=m5[:, :], in_=m5d[b])
                nc.scalar.dma_start(out=a5[:, :], in_=a5d[b])

                spec = []
                for q in range(8):
                    st = specp.tile([128, 4 + T], F16, tag=f"spec{q}", name=f"spec{q}")
                    nc.vector.memset(st[:, 0:4], 0.0)
                    spec.append(st)
                cv5 = r5p.tile([16, 128], F16, tag="cv5", name="cv5")
                ew_chain(a5, None, cv5, [16, 128])
                res5 = r5p.tile([16, 128], F16, tag="res5", name="res5")
                nc.vector.scalar_tensor_tensor(
                    out=res5, in0=m5, scalar=1.0, in1=cv5,
                    op0=ALU.mult, op1=ALU.mult,
                )

                emitted = 0
                for si, (s, W) in enumerate(SLICES):
                    cs = slice(4 + s, 4 + s + W)
                    # one input DMA per tensor per slice: the host layout is
                    # slice-blocked chunk-major, so each transfer is [128, 4W]
                    # with a 2-4 KB contiguous run per partition
                    sam, smm = tiles[bi * len(SLICES) + si]
                    if b == 0 and si == 0:
                        # PE DVFS warmup (discarded results)
                        WW = min(256, W)
                        for i in range(12):
                            wp = psump.tile([128, 256], F32, tag="ps",
                                            name="warm")
                            nc.tensor.matmul(
                                out=wp[:, 0:WW],
                                lhsT=sam[:, 0, 0:128],
                                rhs=sam[:, 0, 0:WW],
                                start=True, stop=True,
                            )

                    # interleave: chain_q emits ahead of mul_{q-1} so the ACT
                    # sins pipeline behind the DVE chains, and spec[q] tiles
                    # complete in q order without head-of-line blocking
                    svs, cvs = [], []

                    def chain(q):
                        sv = workp.tile([128, W], F16, tag="sv", name="sv",
                                        bufs=6)
                        cv = workp.tile([128, W], F16, tag="cv", name="cv",
                                        bufs=6)
                        ew_chain(sam[:, q, :], sv, cv, [128, W])
                        svs.append(sv)
                        cvs.append(cv)

                    def muls(q):
                        mt = smm[:, q, :]
                        # real chunk q
                        nc.vector.scalar_tensor_tensor(
                            out=spec[q][:, cs], in0=mt, scalar=1.0, in1=cvs[q],
                            op0=ALU.mult, op1=ALU.mult,
                        )
                        # imag chunk 4+q (for q=0 partition 0 gets garbage,
                        # patched below with the real row-512 values)
                        nc.vector.scalar_tensor_tensor(
                            out=spec[4 + q][:, cs], in0=mt, scalar=1.0,
                            in1=svs[q], op0=ALU.mult, op1=ALU.mult,
                        )

                    chain(0)
                    chain(1)
                    muls(0)
                    chain(2)
                    issue_input(bi * len(SLICES) + si + 1)
                    muls(1)
                    chain(3)
                    muls(2)
                    muls(3)
                    # patch spec[4] partition 0 for this slice from res5
                    # ([16,128] reshape of the row: sample 128p+i <-> (p, i))
                    nc.scalar.dma_start(
                        out=spec[4][0:1, cs],
                        in_=res5[s // 128 : (s + W) // 128, :],
                    )

                    hi = min(16, (s + W - 132) // 128 + 1)
                    for ct in range(emitted, hi):
                        emit_group(b, spec, ct)
                    emitted = hi

                for ct in range(emitted, 16):
                    emit_group(b, spec, ct)
    nc.compile()
    return nc


_CACHE = {}


def _get_nc():
    if "nc" not in _CACHE:
        _CACHE["nc"] = build_nc()
    return _CACHE["nc"]


def make_in_maps(mag, angle, invbasis):
    """Host-side input marshalling: shard over cores, fp16-convert, interleave
    mag/angle into the slice-blocked chunk-major [128, 4*2048] device layout,
    split out row 512, and pre-pack invbasis to the 1024-row fp16 layout."""
    mag = np.asarray(mag, dtype=np.float32).astype(np.float16)
    angle = np.asarray(angle, dtype=np.float32).astype(np.float16)
    invbasis = np.asarray(invbasis, dtype=np.float32)
    invb_packed = np.ascontiguousarray(
        np.concatenate([invbasis[:513], invbasis[514:1025]], axis=0).astype(np.float16)
    )

    def interleave(x):  # [513, 2048] -> [128, 4*2048] slice-blocked
        m = x[:512].reshape(4, 128, T).transpose(1, 0, 2)  # [128, 4, 2048]
        return np.concatenate(
            [m[:, :, s : s + W].reshape(128, -1) for s, W in SLICES], axis=1
        )

    maps = []
    for i in range(N_CORES):
        mm_ = np.stack([interleave(mag[B_LOCAL * i + b]) for b in range(B_LOCAL)])
        aa_ = np.stack([interleave(angle[B_LOCAL * i + b]) for b in range(B_LOCAL)])
        m5_ = np.ascontiguousarray(
            mag[B_LOCAL * i : B_LOCAL * (i + 1), 512, :].reshape(B_LOCAL, 16, 128)
        )
        a5_ = np.ascontiguousarray(
            angle[B_LOCAL * i : B_LOCAL * (i + 1), 512, :].reshape(B_LOCAL, 16, 128)
        )
        maps.append(
            {
                "mag": np.ascontiguousarray(mm_),
                "angle": np.ascontiguousarray(aa_),
                "mag5": m5_,
                "angle5": a5_,
                "invbasis": invb_packed,
            }
        )
    return maps


def kernel(mag, angle, invbasis, _trace=False, **_ignored):
    nc = _get_nc()
    in_maps = make_in_maps(mag, angle, invbasis)
    res = run_bass_kernel_spmd(nc, in_maps, list(range(N_CORES)), trace=_trace)
    outs = [res.results[i]["out"] for i in range(N_CORES)]
    full = np.concatenate(outs, axis=0).astype(np.float32).reshape(16, 1, 523008)
    if _trace:
        return full, res
    return full


# revision 46
# speedup vs baseline: 1.0025x; 1.0025x over previous
# ISTFT kernel for Trainium2 (8 NeuronCores, data-parallel over batch).
#
# Math: out[b, s] for s = 256*c + r (chunk c, offset r) is
#   out[c, r] = sum_{j=0..3} sum_C spec[C, c-j] * invbasis[C, 256*j + r]
# i.e. the overlap-add is folded into 4 shifted matmuls accumulating in PSUM.
# invbasis rows 513 (imag DC) and 1025 (imag Nyquist) are exactly zero, so the
# contraction packs to exactly 1024 = 8 chunks of 128 partitions:
#   packed rows 0..511    = real rows 0..511   (mag*cos(angle))
#   packed row 512        = real row 512 (Nyquist)
#   packed rows 513..1023 = imag freqs 1..511  (mag*sin(angle))
#
# All-fp16 datapath (same PE rate as bf16, 3 more mantissa bits, DVE 2x):
#   tb  = a*(1/2pi) + 1536        (fp16 magic rounding, ulp(1536)=1)
#   kk  = tb - 1536               (= round(a/2pi), exact small int)
#   red = a - 2pi*kk              (scalar_tensor_tensor, f32 internal)
#   sin = Sin(red*s)              (ACT; s=0.999999 keeps +-pi in the table)
#   cos = Sin(pi/2*s - s*|red|)   (|red| via fp16 sign-bit mask)
#   spec = (1*mag) * {sin,cos}    (scalar_tensor_tensor, fp16 2x)
# Row 512 (real-only) is computed once per batch up-front on a [16,128]
# reshape of the 2048-sample row, then patched into spec[4] partition 0 with
# small per-slice SBUF->SBUF DMAs.
#
# DMA: the host pre-interleaves mag/angle into a slice-blocked [128, 4*2048]
# layout (chunk-major within each column slice) so every per-slice input
# transfer is ONE DMA with 2-4 KB contiguous runs per partition -- ~full HBM
# efficiency and minimal ring latency. invbasis rides the gpsimd SWDGE ring
# and row-512/patches ride the scalar ring, so neither the sync input ring
# nor the ACT queue ever head-of-line blocks (those stalls cost v3/v4 ~10 us).
# A short burst of discarded matmuls on the first angle tile warms the PE
# clock (it runs at ~half speed for the first ~3 us of activity).
import numpy as np

import concourse.bacc as bacc
import concourse.mybir as mybir
import concourse.tile as tile
from concourse.bass_utils import run_bass_kernel_spmd

F32 = mybir.dt.float32
F16 = mybir.dt.float16
U16 = mybir.dt.uint16
ALU = mybir.AluOpType
ACTF = mybir.ActivationFunctionType

TWO_PI = 6.283185307179586
INV_2PI = 1.0 / TWO_PI
PI = 3.141592653589793
MAGIC16 = 1536.0  # 1.5 * 2**10: fp16 ulp 1 -> forces round-to-nearest-int
SIN_SCALE = 0.999999
COS_BIAS = PI / 2 * SIN_SCALE

B_LOCAL = 2      # batches per core (16 total / 8 cores)
T = 2048         # STFT frames
SLICES = [(0, 256), (256, 256), (512, 512), (1024, 512), (1536, 384), (1920, 128)]
N_CORES = 8


def _reg_const(nc, value, dtype=F32):
    key = (dtype, value)
    if key in nc.const_aps.aps:
        return
    t = nc.alloc_sbuf_tensor(f"const-{dtype.name}-{value}", [128, 1], dtype)
    nc.gpsimd.memset(t.ap(), value)
    nc.const_aps.aps[key] = t.ap()


def build_nc(repeat=1):
    nc = bacc.Bacc(target_bir_lowering=False)
    _reg_const(nc, COS_BIAS)
    nc.all_engine_barrier()
    mag = nc.declare_dram_parameter("mag", [B_LOCAL, 128, 4 * T], F16, isOutput=False)
    ang = nc.declare_dram_parameter("angle", [B_LOCAL, 128, 4 * T], F16, isOutput=False)
    m5d = nc.declare_dram_parameter("mag5", [B_LOCAL, 16, 128], F16, isOutput=False)
    a5d = nc.declare_dram_parameter("angle5", [B_LOCAL, 16, 128], F16, isOutput=False)
    invb = nc.declare_dram_parameter("invbasis", [1024, 1024], F16, isOutput=False)
    out = nc.declare_dram_parameter("out", [B_LOCAL, 523008], F16, isOutput=True)

    with tile.TileContext(nc) as tc:
        with (
            tc.tile_pool(name="const", bufs=1) as constp,
            tc.tile_pool(name="stage", bufs=4) as stagep,
            tc.tile_pool(name="spec", bufs=2) as specp,
            tc.tile_pool(name="work", bufs=3) as workp,
            tc.tile_pool(name="r5", bufs=2) as r5p,
            tc.tile_pool(name="osb", bufs=4) as osbp,
            tc.tile_pool(name="psum", bufs=8, space="PSUM") as psump,
        ):
            ib = [
                constp.tile([128, 1024], F16, tag=f"ib{q}", name=f"ib{q}")
                for q in range(8)
            ]
            for q in range(8):
                nc.gpsimd.dma_start(
                    out=ib[q][:, :], in_=invb[128 * q : 128 * (q + 1), :]
                )

            def ew_chain(at, sv_out, cv_out, shp):
                tb = workp.tile(shp, F16, tag="tb", name="tb")
                nc.vector.tensor_scalar(
                    out=tb, in0=at, scalar1=INV_2PI, scalar2=MAGIC16,
                    op0=ALU.mult, op1=ALU.add,
                )
                kk = workp.tile(shp, F16, tag="kk", name="kk")
                nc.vector.tensor_scalar_sub(kk, tb, MAGIC16)
                red = workp.tile(shp, F16, tag="red", name="red", bufs=6)
                nc.vector.scalar_tensor_tensor(
                    out=red, in0=kk, scalar=-TWO_PI, in1=at,
                    op0=ALU.mult, op1=ALU.add,
                )
                ared = workp.tile(shp, F16, tag="ared", name="ared", bufs=6)
                nc.vector.tensor_scalar(
                    out=ared.bitcast(U16), in0=red.bitcast(U16),
                    scalar1=0x7FFF, scalar2=None, op0=ALU.bitwise_and,
                )
                if sv_out is not None:
                    nc.scalar.activation(sv_out, red, ACTF.Sin, scale=SIN_SCALE)
                nc.scalar.activation(cv_out, ared, ACTF.Sin, scale=-SIN_SCALE,
                                     bias=COS_BIAS)

            def emit_group(b, spec, ct):
                """32 accumulating matmuls -> one [128,256] output tile -> DRAM."""
                ps = psump.tile([128, 256], F32, tag="ps", name="ps")
                c0 = 128 * ct
                mmi = 0
                for q in range(8):
                    for j in range(4):
                        nc.tensor.matmul(
                            out=ps[:, :],
                            lhsT=spec[q][:, c0 - j + 4 : c0 - j + 132],
                            rhs=ib[q][:, 256 * j : 256 * (j + 1)],
                            start=(mmi == 0),
                            stop=(mmi == 31),
                        )
                        mmi += 1
                ob = osbp.tile([128, 256], F16, tag="ob", name="ob")
                lo = 256 * (128 * ct - 4)
                if ct >= 14:
                    # tail groups: split drain across ACT+DVE and the output
                    # transfer across both HWDGE rings so the post-matmul
                    # drain latency halves
                    nc.scalar.activation(ob[0:64, :], ps[0:64, :], ACTF.Copy)
                    nc.vector.tensor_scalar_mul(ob[64:128, :], ps[64:128, :], 1.0)
                    hi_rows = 128 if ct < 15 else 127
                    nc.sync.dma_start(out=out[b, lo : lo + 16384],
                                      in_=ob[0:64, :])
                    nc.scalar.dma_start(
                        out=out[b, lo + 16384 : lo + 256 * hi_rows],
                        in_=ob[64:hi_rows, :])
                else:
                    if ct % 2 == 0:
                        nc.scalar.activation(ob[:, :], ps[:, :], ACTF.Copy)
                    else:
                        nc.vector.tensor_scalar_mul(ob[:, :], ps[:, :], 1.0)
                    eng = nc.sync if ct % 2 == 0 else nc.scalar
                    if ct == 0:
                        eng.dma_start(out=out[b, 0:31744], in_=ob[4:128, :])
                    else:
                        eng.dma_start(out=out[b, lo : lo + 32768], in_=ob[:, :])

            # sequenced input stream: each slice's transfer is gated on the
            # previous slice's angle tile through a 1-element DVE copy (WAW on
            # the tile corner). DMA engines fair-share bandwidth over all
            # queued transfers, so without this the first slice completes
            # only after ~its share of the whole 4.2 MB stream; chained, it
            # gets the full ring and lands ~6 us earlier. One-slice lookahead
            # keeps the stream ahead of compute.
            batches = [b for _ in range(repeat) for b in range(B_LOCAL)]
            seq = [(bi, si) for bi in range(len(batches))
                   for si in range(len(SLICES))]
            tiles = {}
            prev_sam = [None]

            def issue_input(k):
                if k >= len(seq):
                    return
                bi, si = seq[k]
                s2, W2 = SLICES[si]
                sam = stagep.tile([128, 4, W2], F16, tag="sam", name="sam",
                                  bufs=4)
                smm = stagep.tile([128, 4, W2], F16, tag="smm", name="smm",
                                  bufs=4)
                if prev_sam[0] is not None:
                    nc.vector.tensor_scalar_mul(
                        sam[0:1, 0:1, 0:1], prev_sam[0][0:1, 0:1, 0:1], 1.0)
                bb = batches[bi]
                if k <= 1:
                    # ramp slices: split each tensor into chunk-pair halves.
                    # Per-transfer DMA bandwidth is engine-fanout-limited
                    # (~83 GB/s), so two co-flowing halves land ~2x sooner
                    # and chain(0)/chain(1) only depend on the first half.
                    # (Halving ALL slices, or quarter-splitting slice 0,
                    # regresses: trigger issue is ~700 ns per 128-descriptor
                    # DMA and contends with the rest of the stream.)
                    h = 2 * W2
                    nc.sync.dma_start(out=sam[:, 0:2, :],
                                      in_=ang[bb, :, 4 * s2 : 4 * s2 + h])
                    nc.sync.dma_start(out=sam[:, 2:4, :],
                                      in_=ang[bb, :, 4 * s2 + h : 4 * s2 + 2 * h])
                    nc.sync.dma_start(out=smm[:, 0:2, :],
                                      in_=mag[bb, :, 4 * s2 : 4 * s2 + h])
                    nc.sync.dma_start(out=smm[:, 2:4, :],
                                      in_=mag[bb, :, 4 * s2 + h : 4 * s2 + 2 * h])
                else:
                    nc.sync.dma_start(out=sam[...], in_=ang[bb, :, 4 * s2 : 4 * (s2 + W2)])
                    nc.sync.dma_start(out=smm[...], in_=mag[bb, :, 4 * s2 : 4 * (s2 + W2)])
                tiles[k] = (sam, smm)
                prev_sam[0] = sam

            issue_input(0)

            for bi, b in enumerate(batches):
                # row-512 inputs: tiny transfers on the scalar ring
                m5 = r5p.tile([16, 128], F16, tag="m5", name="m5")
                a5 = r5p.tile([16, 128], F16, tag="a5", name="a5")
                nc.scalar.dma_start(out=m5[:, :], in_=m5d[b])
                nc.scalar.dma_start(out=a5[:, :], in_=a5d[b])

                spec = []
                for q in range(8):
                    st = specp.tile([128, 4 + T], F16, tag=f"spec{q}", name=f"spec{q}")
                    nc.vector.memset(st[:, 0:4], 0.0)
                    spec.append(st)
                cv5 = r5p.tile([16, 128], F16, tag="cv5", name="cv5")
                ew_chain(a5, None, cv5, [16, 128])
                res5 = r5p.tile([16, 128], F16, tag="res5", name="res5")
                nc.vector.scalar_tensor_tensor(
                    out=res5, in0=m5, scalar=1.0, in1=cv5,
                    op0=ALU.mult, op1=ALU.mult,
                )

                emitted = 0
                for si, (s, W) in enumerate(SLICES):
                    cs = slice(4 + s, 4 + s + W)
                    # one input DMA per tensor per slice: the host layout is
                    # slice-blocked chunk-major, so each transfer is [128, 4W]
                    # with a 2-4 KB contiguous run per partition
                    sam, smm = tiles[bi * len(SLICES) + si]
                    if b == 0 and si == 0:
                        # PE DVFS warmup (discarded results)
                        WW = min(256, W)
                        for i in range(12):
                            wp = psump.tile([128, 256], F32, tag="ps",
                                            name="warm")
                            nc.tensor.matmul(
                                out=wp[:, 0:WW],
                                lhsT=sam[:, 0, 0:128],
                                rhs=sam[:, 0, 0:WW],
                                start=True, stop=True,
                            )

                    # interleave: chain_q emits ahead of mul_{q-1} so the ACT
                    # sins pipeline behind the DVE chains, and spec[q] tiles
                    # complete in q order without head-of-line blocking
                    svs, cvs = [], []

                    def chain(q):
                        sv = workp.tile([128, W], F16, tag="sv", name="sv",
                                        bufs=6)
                        cv = workp.tile([128, W], F16, tag="cv", name="cv",
                                        bufs=6)
                        ew_chain(sam[:, q, :], sv, cv, [128, W])
                        svs.append(sv)
                        cvs.append(cv)

                    def muls(q):
                        mt = smm[:, q, :]
                        # real chunk q
                        nc.vector.scalar_tensor_tensor(
                            out=spec[q][:, cs], in0=mt, scalar=1.0, in1=cvs[q],
                            op0=ALU.mult, op1=ALU.mult,
                        )
                        # imag chunk 4+q (for q=0 partition 0 gets garbage,
                        # patched below with the real row-512 values)
                        nc.vector.scalar_tensor_tensor(
                            out=spec[4 + q][:, cs], in0=mt, scalar=1.0,
                            in1=svs[q], op0=ALU.mult, op1=ALU.mult,
                        )

                    chain(0)
                    chain(1)
                    muls(0)
                    chain(2)
                    issue_input(bi * len(SLICES) + si + 1)
                    muls(1)
                    chain(3)
                    muls(2)
                    muls(3)
                    # patch spec[4] partition 0 for this slice from res5
                    # ([16,128] reshape of the row: sample 128p+i <-> (p, i))
                    nc.scalar.dma_start(
                        out=spec[4][0:1, cs],
                        in_=res5[s // 128 : (s + W) // 128, :],
                    )

                    hi = min(16, (s + W - 132) // 128 + 1)
                    for ct in range(emitted, hi):
                        emit_group(b, spec, ct)
                    emitted = hi

                for ct in range(emitted, 16):
                    emit_group(b, spec, ct)
    nc.compile()
    return nc


_CACHE = {}


def _get_nc():
    if "nc" not in _CACHE:
        _CACHE["nc"] = build_nc()
    return _CACHE["nc"]


def make_in_maps(mag, angle, invbasis):
    """Host-side input marshalling: shard over cores, fp16-convert, interleave
    mag/angle into the slice-blocked chunk-major [128, 4*2048] device layout,
    split out row 512, and pre-pack invbasis to the 1024-row fp16 layout."""
    mag = np.asarray(mag, dtype=np.float32).astype(np.float16)
    angle = np.asarray(angle, dtype=np.float32).astype(np.float16)
    invbasis = np.asarray(invbasis, dtype=np.float32)
    invb_packed = np.ascontiguousarray(
        np.concatenate([invbasis[:513], invbasis[514:1025]], axis=0).astype(np.float16)
    )

    def interleave(x):  # [513, 2048] -> [128, 4*2048] slice-blocked
        m = x[:512].reshape(4, 128, T).transpose(1, 0, 2)  # [128, 4, 2048]
        return np.concatenate(
            [m[:, :, s : s + W].reshape(128, -1) for s, W in SLICES], axis=1
        )

    maps = []
    for i in range(N_CORES):
        mm_ = np.stack([interleave(mag[B_LOCAL * i + b]) for b in range(B_LOCAL)])
        aa_ = np.stack([interleave(angle[B_LOCAL * i + b]) for b in range(B_LOCAL)])
        m5_ = np.ascontiguousarray(
            mag[B_LOCAL * i : B_LOCAL * (i + 1), 512, :].reshape(B_LOCAL, 16, 128)
        )
        a5_ = np.ascontiguousarray(
            angle[B_LOCAL * i : B_LOCAL * (i + 1), 512, :].reshape(B_LOCAL, 16, 128)
        )
        maps.append(
            {
                "mag": np.ascontiguousarray(mm_),
                "angle": np.ascontiguousarray(aa_),
                "mag5": m5_,
                "angle5": a5_,
                "invbasis": invb_packed,
            }
        )
    return maps


def kernel(mag, angle, invbasis, _trace=False, **_ignored):
    nc = _get_nc()
    in_maps = make_in_maps(mag, angle, invbasis)
    res = run_bass_kernel_spmd(nc, in_maps, list(range(N_CORES)), trace=_trace)
    outs = [res.results[i]["out"] for i in range(N_CORES)]
    full = np.concatenate(outs, axis=0).astype(np.float32).reshape(16, 1, 523008)
    if _trace:
        return full, res
    return full
